# revision 65
# baseline (speedup 1.0000x reference)
"""LSTM decoder (nn_Decoder) on 8 Trainium2 NeuronCores.

Strategy:
  - Replicate the sequential LSTM recurrence on all 8 cores (it is serial in T;
    batch=32 gives too little parallelism to shard profitably), shard the output
    head over the vocab dim: each core computes logits[:, :, c*4000:(c+1)*4000].
    Unshard = host-side concat; no device collectives.
  - Recurrence matmuls: out = gates[B=32, 4H] with stationary h^T [K,32]
    replicated into the PE array's 4 column groups (tile_position=(0,32j)) so 4
    weight streams run concurrently -> ~4x util at M=32.
  - h-matmuls are nh-major (all K-chunks for gate cols 0:512 first, then
    512:1024) so the sigmoid(i,f) activation (PSUM bank A) overlaps the
    second half of the h-matmuls (bank B).
  - Embedding projection (e @ W_ih^T) folded into the recurrence as 4 extra
    K-chunks of the same accumulation (they do not depend on h_t, so they
    pipeline into the previous step's elementwise tail).
  - Gate layout on PSUM: partition = 32j+b, free = gate*256+u (hidden j*256+u)
    -> full 128-lane elementwise; h returned to h^T layout by 2 PE transposes.
  - HEAD IS INTERLEAVED INTO THE RECURRENCE (the PE executes in program
    order, so a separate head phase would leave the recurrence's per-step
    dependency stalls as dead PE time and pay the whole head at the end).
    hidden states live in a 4-slot ring of m-tile slices [128, KH*128] bf16;
    head m-tile mt (tokens mt*128..) needs only steps 4mt..4mt+3, and is
    emitted as 2 vocab n-tiles per step during steps 4(mt+1)..4(mt+1)+3.
    The 2 n-tiles (~3.4us of PE streaming) sit between the e-part of t+1 and
    the transpose tail of t, so the PE reaches the transposes after the DVE
    elementwise chain has finished -> no PE stall at the handoff.
  - All weight DMAs (incl. the 8MB W_out shard) are issued up front so the
    head never waits on HBM.
  - Head: per m-tile the 8 vocab n-tiles accumulate in PSUM, bias-add into
    [128, 2000] SBUF staging halves (bf16), one contiguous DMA per half.
  - All matmuls bf16 inputs / fp32 PSUM; c, gates elementwise in fp32.

Host does only data movement / layout prep plus the h0 projection
(z @ W_h.T: 8.4 MFLOP of 320 GFLOP total) and the embedding gather.
"""

import numpy as np
import ml_dtypes
from contextlib import ExitStack

import concourse.bass as bass  # noqa: F401
import concourse.tile as tile
import concourse.bacc as bacc
import concourse.mybir as mybir
from concourse import bass_utils

BF16 = ml_dtypes.bfloat16
N_CORES = 8
B, T = 32, 128
VOCAB, D_EMB, Z_DIM, HID = 32000, 512, 256, 1024
VSH = VOCAB // N_CORES    # 4000 vocab per core
NTOK = B * T              # 4096 tokens; token index = t*32 + b
KH = HID // 128           # 8 hidden K-chunks
KE = D_EMB // 128         # 4 embedding K-chunks
KC = KH + KE              # 12
GW = 4 * HID              # 4096 gate width
NT_HEAD = 8               # vocab tiles per core in the head
NV = VSH // NT_HEAD       # 500
MT_HEAD = NTOK // 128     # 32 token tiles in the head
EPT = 8                   # steps per embedding prefetch tile
SLOT = KH * 128           # ring-slot width: col = k*128 + (t%4)*32 + b
TSH = T // N_CORES        # 16 steps of x_gates precomputed per core
NGATH = 8                 # chunked AllGathers (2 local steps each): the
                          # first chunk completes ~35us in, off the
                          # recurrence's critical path

_NC_CACHE = {}


def _perm():
    # psum gate order n = j*1024 + gate*256 + u  ->  torch W column gate*1024 + j*256 + u
    j = np.arange(4)[:, None, None]
    gate = np.arange(4)[None, :, None]
    u = np.arange(256)[None, None, :]
    return (gate * 1024 + j * 256 + u).reshape(-1)


def _build(repeat=1):
    if repeat in _NC_CACHE:
        return _NC_CACHE[repeat]
    nc = bacc.Bacc("TRN2", debug=False, num_devices=N_CORES)
    dt = mybir.dt
    eTp_d = nc.dram_tensor("eTp", [128, TSH * 128], dt.bfloat16, kind="ExternalInput").ap()
    h0T_d = nc.dram_tensor("h0T", [128, KH * B], dt.bfloat16, kind="ExternalInput").ap()
    Ws_d = nc.dram_tensor("Ws", [128, KC * GW], dt.bfloat16, kind="ExternalInput").ap()
    biasg_d = nc.dram_tensor("bias_g", [1, GW + 32], dt.bfloat16, kind="ExternalInput").ap()
    id_d = nc.dram_tensor("ident", [128, 128], dt.bfloat16, kind="ExternalInput").ap()
    WoT_d = nc.dram_tensor("WoT", [128, NT_HEAD * KH * NV], dt.bfloat16, kind="ExternalInput").ap()
    biaso_d = nc.dram_tensor("bias_o", [128, VSH], dt.bfloat16, kind="ExternalInput").ap()
    out_d = nc.dram_tensor("out", [NTOK, VSH], dt.bfloat16, kind="ExternalOutput").ap()

    with tile.TileContext(nc) as tc, ExitStack() as ctx:
        pers = ctx.enter_context(tc.tile_pool(name="pers", bufs=1))
        ident = pers.tile([128, 128], dt.bfloat16)
        for rep in range(repeat):
            _emit_body(nc, tc, ident, id_d if rep == 0 else None,
                       eTp_d, h0T_d, Ws_d, biasg_d, WoT_d, biaso_d, out_d)
    nc.compile()
    _NC_CACHE[repeat] = nc
    return nc


def _emit_body(nc, tc, ident, id_d, eTp_d, h0T_d, Ws_d, biasg_d, WoT_d, biaso_d, out_d):
    dt = mybir.dt
    ACT = mybir.ActivationFunctionType
    with ExitStack() as rctx:
        wpool = rctx.enter_context(tc.tile_pool(name="ws", bufs=1))
        # Small latency-critical transfers first, then the ~21MB of weights
        # split per-chunk and alternated across BOTH HWDGE queues (SP +
        # Activation) so the recurrence's first steps aren't gated on one
        # serial 12.6MB stream.
        bias_g = wpool.tile([1, GW + 32], dt.bfloat16)
        nc.sync.dma_start(bias_g[:], biasg_d)
        h0T_s = wpool.tile([128, KH * B], dt.bfloat16)
        nc.sync.dma_start(h0T_s[:], h0T_d)
        # pre-pass-scoped tiles: W_ih chunks + embedding prefetch + psum->sbuf
        # staging. The pool closes after the pre-pass, returning ~42KB/part
        # to the recurrence-phase pools.
        pre_ctx = ExitStack()
        wse_pool = pre_ctx.enter_context(tc.tile_pool(name="wse", bufs=1))
        epool = pre_ctx.enter_context(tc.tile_pool(name="eT", bufs=2))
        # 8 staging bufs: a psum->sbuf copy never waits on an SWDGE store
        # completion (~10us latency) of an earlier step.
        xgs = pre_ctx.enter_context(tc.tile_pool(name="xgs", bufs=8))
        xpsum = pre_ctx.enter_context(tc.tile_pool(name="xps", bufs=3, space="PSUM"))
        ws_e = wse_pool.tile([128, KE * GW], dt.bfloat16)  # 32KB/part
        # both embedding tiles up front: their descriptors must not queue
        # behind the weight streams (the HW round-robins sub-queues, but
        # per-sub-queue order still matters).
        ets = []
        for h in range(2):
            et_h = epool.tile([128, EPT * 128], dt.bfloat16, tag="et",
                              name=f"et{h}")
            nc.sync.dma_start(et_h[:], eTp_d[:, h * EPT * 128:(h + 1) * EPT * 128])
            ets.append(et_h)
        if id_d is not None:
            # identity for the PE transposes: small, needed at step 0's tail.
            nc.sync.dma_start(ident[:], id_d)
        # W_ih chunks first on both HWDGE queues: the pre-pass starts ~10us
        # in; W_hh then arrives progressively under it, W_out after.
        for cix in range(KE):
            eng = nc.scalar if cix % 2 == 0 else nc.sync
            eng.dma_start(ws_e[:, cix * GW:(cix + 1) * GW],
                          Ws_d[:, (KH + cix) * GW:(KH + cix + 1) * GW])
        ws = wpool.tile([128, KH * GW], dt.bfloat16)  # 64KB/part (W_hh)
        for k in range(KH):
            eng = nc.scalar if k % 2 == 0 else nc.sync
            eng.dma_start(ws[:, k * GW:(k + 1) * GW],
                          Ws_d[:, k * GW:(k + 1) * GW])
        wo = wpool.tile([128, NT_HEAD * KH * NV], dt.bfloat16)  # 62.5KB/part
        for nt in range(NT_HEAD):
            eng = nc.scalar if nt % 2 == 0 else nc.sync
            eng.dma_start(
                wo[:, nt * KH * NV:(nt + 1) * KH * NV],
                WoT_d[:, nt * KH * NV:(nt + 1) * KH * NV])
        bias_o = wpool.tile([128, VSH], dt.bfloat16)  # 7.8KB/part
        nc.scalar.dma_start(bias_o[:], biaso_d)

        # x_gates sharding: this core precomputes e@W_ih^T + b for its own
        # TSH steps, the 8 cores exchange shards with NGATH chunked
        # HBM AllGathers that pipeline ahead of the recurrence's consumption.
        dram = rctx.enter_context(tc.tile_pool(name="xgd", bufs=1, space="DRAM"))
        GSTEP = TSH // NGATH  # local steps per gather chunk
        # [step-block, 1024] row layout: every 256KB store/fetch is one
        # fully contiguous DRAM block (the [128, steps*1024] layout made
        # each transfer 128 strided 2KB lines -> ~10us per SWDGE store).
        xg_loc = [dram.tile([GSTEP * 128, 1024], dt.bfloat16, tag=f"xl{q}",
                            name=f"xg_loc{q}") for q in range(NGATH)]
        xg_all = [dram.tile([N_CORES * GSTEP * 128, 1024], dt.bfloat16,
                            tag=f"xa{q}", name=f"xg_all{q}",
                            addr_space="Shared") for q in range(NGATH)]

        def emit_e_part(g, t, et, stop_last=False):
            # gate bias as a K=1 rank-1 matmul (ones stationary, bias row
            # streamed) -- keeps the bias add off the DVE critical chain.
            for nh in range(2):
                for j in range(4):
                    nc.tensor.matmul(
                        g[32 * j:32 * j + 32, nh * 512:(nh + 1) * 512],
                        bias_g[:, GW:GW + 32],
                        bias_g[:, j * 1024 + nh * 512:j * 1024 + (nh + 1) * 512],
                        start=True, stop=False,
                        tile_position=(0, 32 * j),
                    )
            # e-chunks of the pre-pass step.
            for nh in range(2):
                for cix in range(KE):
                    off = (t % EPT) * 128 + cix * 32
                    for j in range(4):
                        nc.tensor.matmul(
                            g[32 * j:32 * j + 32, nh * 512:(nh + 1) * 512],
                            et[:, off:off + 32],
                            ws_e[:, cix * GW + j * 1024 + nh * 512:
                                 cix * GW + j * 1024 + (nh + 1) * 512],
                            start=False, stop=(stop_last and cix == KE - 1),
                            tile_position=(0, 32 * j),
                        )

        def emit_inject(g, xr):
            # inject the prefetched x_gates row-block into the gate PSUM as
            # one K=128 full-identity matmul per nh half (out = I.T @ xr =
            # xr): 512 streaming cycles each, all 128 partitions at once;
            # the h-matmuls then accumulate on top.
            for nh in range(2):
                nc.tensor.matmul(
                    g[:, nh * 512:(nh + 1) * 512],
                    ident[:],
                    xr[:, nh * 512:(nh + 1) * 512],
                    start=True, stop=False,
                )

        def xg_fetch(t):
            # prefetch global step t's x_gates [128, 1024] from the gathered
            # shards: core block t//16, gather chunk (t%16)//4, substep t%4.
            xr = xgr.tile([128, 1024], dt.bfloat16, tag="xr")
            c, q, s = t // TSH, (t % TSH) // GSTEP, t % GSTEP
            row = (c * GSTEP + s) * 128
            nc.sync.dma_start(xr[:], xg_all[q][row:row + 128, :])
            return xr

        def emit_h_part(g, t, slot_rd):
            # nh-major: finish gate cols 0:512 (PSUM bank A) across all
            # K-chunks first so the sigmoid(i,f) can start while the PE
            # still streams cols 512:1024 (bank B).
            for nh in range(2):
                for k in range(KH):
                    if t == 0:
                        lhsT = h0T_s[:, k * 32:(k + 1) * 32]
                    else:
                        # slot col = k*128 + s*32 + b, chunk k = 2j+m.
                        off = k * 128 + ((t - 1) % 4) * 32
                        lhsT = slot_rd[:, off:off + 32]
                    for j in range(4):
                        nc.tensor.matmul(
                            g[32 * j:32 * j + 32, nh * 512:(nh + 1) * 512],
                            lhsT,
                            ws[:, k * GW + j * 1024 + nh * 512:
                               k * GW + j * 1024 + (nh + 1) * 512],
                            start=False, stop=(k == KH - 1),
                            tile_position=(0, 32 * j),
                        )

        def emit_head_nts(mt, slot, nts, stage):
            # head n-tiles for token m-tile mt: logits[mt*128:(mt+1)*128,
            # nt*500:(nt+1)*500] accumulated over the 8 hidden K-chunks.
            for nt in nts:
                ps = hpsum.tile([128, NV], dt.float32, tag="hp")
                for k in range(KH):
                    nc.tensor.matmul(
                        ps[:],
                        slot[:, k * 128:(k + 1) * 128],
                        wo[:, nt * KH * NV + k * NV:nt * KH * NV + (k + 1) * NV],
                        start=(k == 0), stop=(k == KH - 1),
                    )
                nc.vector.tensor_add(
                    stage[:, (nt % 2) * NV:(nt % 2 + 1) * NV], ps[:],
                    bias_o[:, nt * NV:(nt + 1) * NV])
                if nt % 2 == 1:
                    nc.sync.dma_start(
                        out_d[mt * 128:(mt + 1) * 128,
                              (nt - 1) * NV:(nt + 1) * NV], stage[:])

        def emit_tail_compute(g, t):
            # elementwise tail on ACT/DVE only (no PE instructions), so the
            # PE can stream head n-tiles while this chain runs.
            if_sb = ew.tile([128, 512], dt.float32, tag="if")
            nc.scalar.activation(if_sb[:], g[:, 0:512], ACT.Sigmoid)
            # f*c does not need the nh=1 gates; it runs during the nh=1 MMs.
            for m in range(2):
                nc.vector.tensor_mul(
                    c_sb[:, m * 128:(m + 1) * 128],
                    c_sb[:, m * 128:(m + 1) * 128],
                    if_sb[:, 256 + m * 128:256 + (m + 1) * 128])
            # gg/o/t1/tc/h are single-buffered: their cross-step reuse is
            # ordered by the serial c-chain (DVE in-order) or by framework
            # WAR semaphores on the slack-rich ACT queue.
            gg_sb = ew1.tile([128, 256], dt.float32, tag="gg")
            o_sb = ew1.tile([128, 256], dt.float32, tag="o")
            t1 = ew1.tile([128, 256], dt.float32, tag="t1")
            tc_sb = ew1.tile([128, 256], dt.float32, tag="tc")
            h_bf = ew1.tile([128, 256], dt.bfloat16, tag="h")
            for m in range(2):
                sl = slice(m * 128, (m + 1) * 128)
                nc.scalar.activation(gg_sb[:, sl], g[:, 512 + m * 128:512 + (m + 1) * 128], ACT.Tanh)
                nc.scalar.activation(o_sb[:, sl], g[:, 768 + m * 128:768 + (m + 1) * 128], ACT.Sigmoid)
                nc.vector.tensor_mul(t1[:, sl], if_sb[:, sl], gg_sb[:, sl])
                nc.vector.tensor_add(c_sb[:, sl], c_sb[:, sl], t1[:, sl])
                nc.scalar.activation(tc_sb[:, sl], c_sb[:, sl], ACT.Tanh)
                nc.vector.tensor_mul(h_bf[:, sl], o_sb[:, sl], tc_sb[:, sl])
            return h_bf

        def emit_tail_transposes(t, slot, h_bf):
            slot_v = slot[:].rearrange("p (j m n) -> p j m n", j=4, m=2)
            for m in range(2):
                sl = slice(m * 128, (m + 1) * 128)
                tr = tpsum.tile([128, 128], dt.bfloat16, tag="tr")
                nc.tensor.transpose(tr[:], h_bf[:, sl], ident[:])
                nc.vector.tensor_copy(
                    slot_v[:, :, m, (t % 4) * 32:(t % 4) * 32 + 32],
                    tr[:].rearrange("p (j b) -> p j b", j=4),
                )

        # ---- x_gates pre-pass: this core's TSH steps, 4 chunked AllGathers.
        # The xg_loc stores ride the GpSimd SWDGE queue so they never sit
        # behind the multi-MB weight streams on the HWDGE rings.
        for gix in range(TSH):
            et = ets[gix // EPT]
            xps = xpsum.tile([128, 1024], dt.float32, tag="xp")
            emit_e_part(xps, gix, et, stop_last=True)
            xsb = xgs.tile([128, 1024], dt.bfloat16, tag="xg")
            nc.vector.tensor_copy(xsb[:], xps[:])
            q, s = gix // GSTEP, gix % GSTEP
            nc.gpsimd.dma_start(xg_loc[q][s * 128:(s + 1) * 128, :], xsb[:])
            if s == GSTEP - 1:
                nc.gpsimd.collective_compute(
                    "AllGather", mybir.AluOpType.bypass,
                    replica_groups=[list(range(N_CORES))],
                    ins=[xg_loc[q][:]], outs=[xg_all[q][:]])
        pre_ctx.close()  # frees ws_e/eT/xgs SBUF + the pre-pass PSUM pool

        # ---- recurrence + interleaved head.
        gpsum = rctx.enter_context(tc.tile_pool(name="gps", bufs=2, space="PSUM"))
        tpsum = rctx.enter_context(tc.tile_pool(name="tps", bufs=2, space="PSUM"))
        hpsum = rctx.enter_context(tc.tile_pool(name="hps", bufs=2, space="PSUM"))
        xgr = rctx.enter_context(tc.tile_pool(name="xgr", bufs=4))
        ring = rctx.enter_context(tc.tile_pool(name="ring", bufs=3))
        ew = rctx.enter_context(tc.tile_pool(name="ew", bufs=2))
        ew1 = rctx.enter_context(tc.tile_pool(name="ew1", bufs=1))
        opool = rctx.enter_context(tc.tile_pool(name="osb", bufs=2))
        cpool = rctx.enter_context(tc.tile_pool(name="cst", bufs=1))
        c_sb = cpool.tile([128, 256], dt.float32)
        nc.vector.memset(c_sb[:], 0.0)
        g_cur = None
        slot_cur = None
        slot_prev = None
        stage = None
        xg_tiles = {tt: xg_fetch(tt) for tt in range(3)}
        for t in range(T):
            if t % 4 == 0:
                slot_prev, slot_cur = slot_cur, ring.tile([128, SLOT], dt.bfloat16, tag="hs")
            if t == 0:
                g_cur = gpsum.tile([128, 1024], dt.float32, tag="g")
                emit_inject(g_cur, xg_tiles.pop(0))
            g = g_cur
            emit_h_part(g, t, slot_prev if t % 4 == 0 else slot_cur)
            if t + 1 < T:
                g_cur = gpsum.tile([128, 1024], dt.float32, tag="g")
                emit_inject(g_cur, xg_tiles.pop(t + 1))
            if t + 3 < T:
                # fetched AFTER the inject that last read this ring buffer
                # (bufs=3), so the pool's WAR ordering stays emission-true.
                xg_tiles[t + 3] = xg_fetch(t + 3)
            # head n-tiles for the previous m-tile: 2 per step, one BEFORE
            # the transposes (so the PE reaches them after the DVE chain is
            # done) and one AFTER (so the ring-copy latency hides under it
            # instead of stalling the next step's first h-matmul).
            h_bf = emit_tail_compute(g, t)
            if t >= 4:
                mt = t // 4 - 1
                nts = [2 * (t % 4), 2 * (t % 4) + 1]
                stage = opool.tile([128, 2 * NV], dt.bfloat16, tag="osb")
                emit_head_nts(mt, slot_prev, nts[:1], stage)
                emit_tail_transposes(t, slot_cur, h_bf)
                emit_head_nts(mt, slot_prev, nts[1:], stage)
            else:
                emit_tail_transposes(t, slot_cur, h_bf)
        # last m-tile's head after the loop.
        for pair in range(4):
            stage = opool.tile([128, 2 * NV], dt.bfloat16, tag="osb")
            emit_head_nts(MT_HEAD - 1, slot_cur, [2 * pair, 2 * pair + 1], stage)


def prep_in_maps(z, x, W_h, b_h, emb, W_ih, W_hh, b_ih, b_hh, W_out, b_out):
    f32 = np.float32
    z = np.asarray(z, f32)
    W_h = np.asarray(W_h, f32)
    b_h = np.asarray(b_h, f32)
    emb = np.asarray(emb, f32)
    W_ih = np.asarray(W_ih, f32)
    W_hh = np.asarray(W_hh, f32)
    b_ih = np.asarray(b_ih, f32)
    b_hh = np.asarray(b_hh, f32)
    W_out = np.asarray(W_out, f32)
    b_out = np.asarray(b_out, f32)
    x = np.asarray(x)

    h0 = np.tanh(z @ W_h.T + b_h)                       # [B, H]
    e = emb[x]                                          # [B, T, D]
    # eT[p, t*128 + c*32 + b] = e[b, t, c*128+p]
    eT = e.transpose(2, 1, 0).reshape(KE, 128, T, B)
    eT = np.ascontiguousarray(eT.transpose(1, 2, 0, 3)).reshape(128, T * KE * 32)
    # h0T[p, c*32+b] = h0[b, c*128+p]
    h0T = np.ascontiguousarray(h0.T.reshape(KH, 128, B).transpose(1, 0, 2)).reshape(128, KH * B)
    perm = _perm()
    Wcat = np.concatenate([W_hh.T, W_ih.T], axis=0)     # [H+D, 4H]
    Wp = Wcat[:, perm]
    Ws = np.ascontiguousarray(Wp.reshape(KC, 128, GW).transpose(1, 0, 2)).reshape(128, KC * GW)
    bsum = (b_ih + b_hh)[perm]
    bias_g = np.concatenate([bsum, np.ones(32, np.float32)]).reshape(1, GW + 32)
    ident = np.eye(128, dtype=BF16)

    base = {
        "h0T": h0T.astype(BF16),
        "Ws": Ws.astype(BF16),
        "bias_g": bias_g.astype(BF16),
        "ident": ident,
    }
    eT16 = eT.astype(BF16)
    in_maps = []
    for c in range(N_CORES):
        Wsh = W_out[c * VSH:(c + 1) * VSH]              # [4000, 1024]
        WoT = np.ascontiguousarray(
            Wsh.reshape(NT_HEAD, NV, KH, 128).transpose(3, 0, 2, 1)
        ).reshape(128, NT_HEAD * KH * NV)
        bsh = b_out[c * VSH:(c + 1) * VSH]
        bias_o = np.ascontiguousarray(np.broadcast_to(bsh, (128, VSH)))
        m = dict(base)
        # core c precomputes x_gates for its own steps [c*TSH, (c+1)*TSH)
        m["eTp"] = np.ascontiguousarray(
            eT16[:, c * TSH * 128:(c + 1) * TSH * 128])
        m["WoT"] = WoT.astype(BF16)
        m["bias_o"] = bias_o.astype(BF16)
        in_maps.append(m)
    return in_maps


def assemble(results):
    outs = [np.asarray(r["out"]).astype(np.float32).reshape(T, B, VSH)
            for r in results]
    full = np.concatenate(outs, axis=2)                 # [T, B, VOCAB]
    return np.ascontiguousarray(full.transpose(1, 0, 2))


def kernel(**inputs):
    in_maps = prep_in_maps(**inputs)
    nc = _build()
    res = bass_utils.run_bass_kernel_spmd(nc, in_maps, core_ids=list(range(N_CORES)))
    return assemble(res.results)
